# revision 8
# baseline (speedup 1.0000x reference)
import os

os.environ.setdefault("MYCRO_LOCAL_CACHE", "1")

import numpy as np

B, T, C = 2, 2048, 1024
H, D = 16, 64
WIN = 256
NCORES = 8
CHUNK = 512
HALO = 256
TQ = CHUNK + HALO
P = 128

NQT = CHUNK // P
NKT = TQ // P

MASK_PAIRS = [(0, 0), (1, 1), (2, 2), (3, 3),
              (1, 0),
              (2, 0), (3, 1), (4, 2), (5, 3)]
NMASK = len(MASK_PAIRS)

USE_F32R = True

_MOD = None


def _build_module():
    import concourse.bass as bass
    import concourse.bacc as bacc
    import concourse.mybir as mybir
    import concourse.tile as tile
    from concourse.masks import make_identity
    from contextlib import ExitStack

    F32 = mybir.dt.float32
    F32R = mybir.dt.float32r if USE_F32R else F32

    def rr(ap):
        return ap.bitcast(F32R) if USE_F32R else ap

    nc = bacc.Bacc(
        "TRN2",
        target_bir_lowering=False,
        debug=False,
        enable_asserts=False,
        num_devices=NCORES,
    )

    xh = nc.dram_tensor("xh", [TQ, C], F32, kind="ExternalInput").ap()
    wa = nc.dram_tensor("wa", [C, 3 * C], F32, kind="ExternalInput").ap()
    ba = nc.dram_tensor("ba", [3 * C], F32, kind="ExternalInput").ap()
    wp = nc.dram_tensor("wp", [C, C], F32, kind="ExternalInput").ap()
    bp = nc.dram_tensor("bp", [C], F32, kind="ExternalInput").ap()
    mk = nc.dram_tensor("mk", [P, NMASK, P], F32, kind="ExternalInput").ap()
    y = nc.dram_tensor("y", [CHUNK, C], F32, kind="ExternalOutput").ap()

    Exp = mybir.ActivationFunctionType.Exp
    Ident = mybir.ActivationFunctionType.Identity
    ADD = mybir.AluOpType.add

    with tile.TileContext(nc) as tc, ExitStack() as ctx:
        const = ctx.enter_context(tc.tile_pool(name="const", bufs=1))
        xload = ctx.enter_context(tc.tile_pool(name="xload", bufs=2))
        big = ctx.enter_context(tc.tile_pool(name="big", bufs=1))
        wpool = ctx.enter_context(tc.tile_pool(name="wpool", bufs=8))
        slabp = ctx.enter_context(tc.tile_pool(name="slabp", bufs=8))
        small = ctx.enter_context(tc.tile_pool(name="small", bufs=8))
        yout = ctx.enter_context(tc.tile_pool(name="yout", bufs=3))
        ps512 = ctx.enter_context(tc.tile_pool(name="ps512", bufs=2, space="PSUM"))
        ps384 = ctx.enter_context(tc.tile_pool(name="ps384", bufs=2, space="PSUM"))
        ps128 = ctx.enter_context(tc.tile_pool(name="ps128", bufs=2, space="PSUM"))

        ident = const.tile([P, P], F32)
        make_identity(nc, ident)

        bqk = const.tile([P, 16], F32)
        with nc.allow_non_contiguous_dma(reason="tiny bias rearrange"):
            nc.sync.dma_start(bqk, ba[: 2 * C].rearrange("(j p) -> p j", p=P))

        bv_row = xload.tile([1, C], F32, tag="brow")
        nc.sync.dma_start(bv_row, ba[None, 2 * C:])
        bv_b = const.tile([P, C], F32)
        nc.gpsimd.partition_broadcast(bv_b, bv_row)

        bp_row = xload.tile([1, C], F32, tag="brow")
        nc.sync.dma_start(bp_row, bp[None, :])
        bp_b = const.tile([P, C], F32)
        nc.gpsimd.partition_broadcast(bp_b, bp_row)

        masks = const.tile([P, NMASK, P], F32)
        nc.sync.dma_start(masks, mk)

        xT = big.tile([P, C // P, TQ], F32R, tag="xT")
        for tt in range(NKT):
            xrow = xload.tile([P, C], F32, tag="xrow")
            nc.sync.dma_start(xrow, xh[tt * P:(tt + 1) * P, :])
            for ct in range(C // P):
                pt = ps128.tile([P, P], F32, tag="ps128")
                nc.tensor.transpose(pt, xrow[:, ct * P:(ct + 1) * P], ident)
                if (tt + ct) % 2 == 0:
                    nc.vector.tensor_copy(xT[:, ct, tt * P:(tt + 1) * P], pt)
                else:
                    nc.scalar.activation(xT[:, ct, tt * P:(tt + 1) * P], pt,
                                         Ident, bias=0.0, scale=1.0)

        QT = big.tile([P, 8, CHUNK], F32R, tag="QT")
        KT = big.tile([P, 8, TQ], F32R, tag="KT")
        VS = big.tile([P, NKT, H, D + 2], F32R, tag="VS")
        ones_h = const.tile([P, NKT * H], F32)
        nc.gpsimd.memset(ones_h, 1.0)
        nc.vector.tensor_copy(
            VS[:, :, :, D], ones_h.rearrange("p (t h) -> p t h", h=H))
        nc.vector.tensor_copy(
            VS[:, :, :, D + 1], ones_h.rearrange("p (t h) -> p t h", h=H))

        for jg in range(2):
            wts = []
            for ct in range(C // P):
                wt = wpool.tile([P, 512], F32R, tag="wchunk")
                nc.sync.dma_start(
                    wt, wa[ct * P:(ct + 1) * P,
                           jg * 512:(jg + 1) * 512].bitcast(F32R))
                wts.append(wt)
            for jl in range(4):
                jt = jg * 4 + jl
                ps = ps512.tile([P, CHUNK], F32, tag="ps512")
                for ct in range(C // P):
                    nc.tensor.matmul(
                        ps,
                        wts[ct][:, jl * P:(jl + 1) * P],
                        xT[:, ct, HALO:TQ],
                        start=(ct == 0), stop=(ct == C // P - 1))
                nc.scalar.activation(QT[:, jt, :], ps, Ident,
                                     bias=bqk[:, jt:jt + 1], scale=1.0)

        for vc in range(2):
            wts = []
            for ct in range(C // P):
                wt = wpool.tile([P, 512], F32R, tag="wchunk")
                nc.sync.dma_start(
                    wt, wa[ct * P:(ct + 1) * P,
                           2 * C + vc * 512: 2 * C + (vc + 1) * 512]
                        .bitcast(F32R))
                wts.append(wt)
            for tt in range(NKT):
                ps = ps512.tile([P, 512], F32, tag="ps512")
                for ct in range(C // P):
                    nc.tensor.matmul(
                        ps,
                        xT[:, ct, tt * P:(tt + 1) * P],
                        wts[ct],
                        start=(ct == 0), stop=(ct == C // P - 1))
                nc.vector.tensor_tensor(
                    VS[:, tt, vc * 8:(vc + 1) * 8, 0:D],
                    ps.rearrange("p (h d) -> p h d", d=D),
                    bv_b[:, vc * 512:(vc + 1) * 512]
                        .rearrange("p (h d) -> p h d", d=D),
                    ADD)

        for jg in range(2):
            wts = []
            for ct in range(C // P):
                wt = wpool.tile([P, 512], F32R, tag="wchunk")
                nc.sync.dma_start(
                    wt, wa[ct * P:(ct + 1) * P,
                           C + jg * 512: C + (jg + 1) * 512].bitcast(F32R))
                wts.append(wt)
            for jl in range(4):
                jt = jg * 4 + jl
                for half in range(2):
                    ps = ps384.tile([P, 384], F32, tag="psk")
                    for ct in range(C // P):
                        nc.tensor.matmul(
                            ps,
                            wts[ct][:, jl * P:(jl + 1) * P],
                            xT[:, ct, half * 384:(half + 1) * 384],
                            start=(ct == 0), stop=(ct == C // P - 1))
                    nc.scalar.activation(
                        KT[:, jt, half * 384:(half + 1) * 384], ps, Ident,
                        bias=bqk[:, 8 + jt: 9 + jt], scale=1.0)

        outT = big.tile([P, 8, CHUNK], F32R, tag="outT")
        scale = 1.0 / np.sqrt(D)
        for hp in range(H // 2):
            pair = [small.tile([P, P], F32, tag="pair", name=f"pair{hp}_{i}")
                    for i in range(NQT)]
            for hh in range(2):
                h = 2 * hp + hh
                p0 = hh * 64
                qt_h = QT[p0:p0 + 64, hp, :]
                kt_h = KT[p0:p0 + 64, hp, :]

                slabs = []
                for kt in range(NKT):
                    qlo = max(0, kt - 2)
                    qhi = min(NQT - 1, kt)
                    nq = (qhi - qlo + 1) * P
                    ps = ps384.tile([P, 384], F32, tag="ps384")
                    nc.tensor.matmul(
                        ps[:, :nq],
                        kt_h[:, kt * P:(kt + 1) * P],
                        qt_h[:, qlo * P: qlo * P + nq],
                        start=True, stop=True)
                    slab = slabp.tile([P, 384], F32R, tag="slab")
                    nc.scalar.activation(slab[:, :nq], ps[:, :nq], Exp,
                                         bias=0.0, scale=float(scale))
                    slabs.append(slab)

                for mi, (kt, qt) in enumerate(MASK_PAIRS):
                    qoff = (qt - max(0, kt - 2)) * P
                    nc.vector.tensor_mul(
                        slabs[kt][:, qoff:qoff + P],
                        slabs[kt][:, qoff:qoff + P],
                        masks[:, mi, :])

                for qt in range(NQT):
                    pav = ps128.tile([P, D + 2], F32, tag="ps128")
                    for i, kt in enumerate(range(qt, qt + 3)):
                        qoff = (qt - max(0, kt - 2)) * P
                        nc.tensor.matmul(
                            pav,
                            slabs[kt][:, qoff:qoff + P],
                            VS[:, kt, h, :],
                            start=(i == 0), stop=(i == 2))
                    rcp = small.tile([P, 1], F32, tag="rcp")
                    nc.vector.reciprocal(rcp, pav[:, D:D + 1])
                    nc.vector.tensor_scalar_mul(
                        pair[qt][:, p0:p0 + 64], pav[:, 0:D], rcp)

            for qt in range(NQT):
                pt = ps128.tile([P, P], F32, tag="ps128")
                nc.tensor.transpose(pt, pair[qt], ident)
                nc.vector.tensor_copy(outT[:, hp, qt * P:(qt + 1) * P], pt)

        for oc in range(2):
            wts = []
            for hp in range(8):
                wt = wpool.tile([P, 512], F32R, tag="wchunk")
                nc.sync.dma_start(
                    wt, wp[hp * P:(hp + 1) * P,
                           oc * 512:(oc + 1) * 512].bitcast(F32R))
                wts.append(wt)
            for tb in range(NQT):
                ps = ps512.tile([P, 512], F32, tag="ps512")
                for hp in range(8):
                    nc.tensor.matmul(
                        ps,
                        outT[:, hp, tb * P:(tb + 1) * P],
                        wts[hp],
                        start=(hp == 0), stop=(hp == 7))
                ysb = yout.tile([P, 512], F32, tag="ysb")
                nc.vector.tensor_tensor(
                    ysb, ps, bp_b[:, oc * 512:(oc + 1) * 512], ADD)
                nc.sync.dma_start(
                    y[tb * P:(tb + 1) * P, oc * 512:(oc + 1) * 512], ysb)

    nc.compile()
    return nc


def _get_module():
    global _MOD
    if _MOD is None:
        _MOD = _build_module()
    return _MOD


def _mask_tiles(chunk_start: int) -> np.ndarray:
    out = np.zeros((P, NMASK, P), np.float32)
    kk = np.arange(P)[:, None]
    qq = np.arange(P)[None, :]
    for mi, (kt, qt) in enumerate(MASK_PAIRS):
        key_abs = chunk_start - HALO + kt * P + kk
        q_abs = chunk_start + qt * P + qq
        valid = (key_abs <= q_abs) & (key_abs >= q_abs - WIN) & (key_abs >= 0)
        out[:, mi, :] = valid.astype(np.float32)
    return out


def _in_maps(x, W_attn, b_attn, W_proj, b_proj):
    maps = []
    for c in range(NCORES):
        b, k = divmod(c, NCORES // B)
        t0 = k * CHUNK
        xhalo = np.zeros((TQ, C), np.float32)
        lo = t0 - HALO
        src_lo = max(0, lo)
        xhalo[src_lo - lo:, :] = x[b, src_lo: t0 + CHUNK]
        maps.append({
            "xh": np.ascontiguousarray(xhalo),
            "wa": np.ascontiguousarray(W_attn, np.float32),
            "ba": np.ascontiguousarray(b_attn, np.float32),
            "wp": np.ascontiguousarray(W_proj, np.float32),
            "bp": np.ascontiguousarray(b_proj, np.float32),
            "mk": _mask_tiles(t0),
        })
    return maps


def _run(inputs, trace=False, trace_kwargs=None):
    from concourse import bass_utils

    nc = _get_module()
    maps = _in_maps(**inputs)
    res = bass_utils.run_bass_kernel_spmd(
        nc, maps, core_ids=list(range(NCORES)),
        trace=trace, **(trace_kwargs or {}))
    out = np.empty((B, T, C), np.float32)
    for c in range(NCORES):
        b, k = divmod(c, NCORES // B)
        out[b, k * CHUNK:(k + 1) * CHUNK] = res.results[c]["y"]
    return out, res


def kernel(x, W_attn, b_attn, W_proj, b_proj):
    inputs = dict(x=np.asarray(x, np.float32), W_attn=W_attn, b_attn=b_attn,
                  W_proj=W_proj, b_proj=b_proj)
    out, _ = _run(inputs)
    return out


# revision 9
# speedup vs baseline: 1.3187x; 1.3187x over previous
import os

os.environ.setdefault("MYCRO_LOCAL_CACHE", "1")

import numpy as np

B, T, C = 2, 2048, 1024
H, D = 16, 64
WIN = 256
NCORES = 8
CHUNK = 512
HALO = 256
TQ = CHUNK + HALO
P = 128

NQT = CHUNK // P
NKT = TQ // P

MASK_PAIRS = [(0, 0), (1, 1), (2, 2), (3, 3),
              (1, 0),
              (2, 0), (3, 1), (4, 2), (5, 3)]
NMASK = len(MASK_PAIRS)

SCORE_DT = os.environ.get("KERNEL_SCORE_DT", "bf16")
VALUE_DT = os.environ.get("KERNEL_VALUE_DT", "bf16")

_MOD = None


def _np_dt(name):
    if name == "bf16":
        import ml_dtypes
        return np.dtype(ml_dtypes.bfloat16)
    return np.dtype(np.float32)


def _build_module():
    import concourse.bacc as bacc
    import concourse.mybir as mybir
    import concourse.tile as tile
    from concourse.masks import make_identity
    from contextlib import ExitStack

    F32 = mybir.dt.float32
    SDT = mybir.dt.bfloat16 if SCORE_DT == "bf16" else mybir.dt.float32r
    VDT = mybir.dt.bfloat16 if VALUE_DT == "bf16" else mybir.dt.float32r

    nc = bacc.Bacc(
        "TRN2",
        target_bir_lowering=False,
        debug=False,
        enable_asserts=False,
        num_devices=NCORES,
    )

    xh = nc.dram_tensor("xh", [TQ, C], F32, kind="ExternalInput").ap()
    wa = nc.dram_tensor("wa", [C, 3 * C], SDT, kind="ExternalInput").ap()
    ba = nc.dram_tensor("ba", [3 * C], F32, kind="ExternalInput").ap()
    wp = nc.dram_tensor("wp", [C, C], VDT, kind="ExternalInput").ap()
    bp = nc.dram_tensor("bp", [C], F32, kind="ExternalInput").ap()
    mk = nc.dram_tensor("mk", [P, NMASK, P], VDT, kind="ExternalInput").ap()
    y = nc.dram_tensor("y", [CHUNK, C], F32, kind="ExternalOutput").ap()

    Exp = mybir.ActivationFunctionType.Exp
    Ident = mybir.ActivationFunctionType.Identity
    ADD = mybir.AluOpType.add

    with tile.TileContext(nc) as tc, ExitStack() as ctx:
        const = ctx.enter_context(tc.tile_pool(name="const", bufs=1))
        xload = ctx.enter_context(tc.tile_pool(name="xload", bufs=2))
        big = ctx.enter_context(tc.tile_pool(name="big", bufs=1))
        wpool = ctx.enter_context(tc.tile_pool(name="wpool", bufs=8))
        slabp = ctx.enter_context(tc.tile_pool(name="slabp", bufs=8))
        small = ctx.enter_context(tc.tile_pool(name="small", bufs=8))
        yout = ctx.enter_context(tc.tile_pool(name="yout", bufs=3))
        ps512 = ctx.enter_context(tc.tile_pool(name="ps512", bufs=2, space="PSUM"))
        ps384 = ctx.enter_context(tc.tile_pool(name="ps384", bufs=2, space="PSUM"))
        ps128 = ctx.enter_context(tc.tile_pool(name="ps128", bufs=2, space="PSUM"))

        ident = const.tile([P, P], F32)
        make_identity(nc, ident)

        bqk = const.tile([P, 16], F32)
        with nc.allow_non_contiguous_dma(reason="tiny bias rearrange"):
            nc.sync.dma_start(bqk, ba[: 2 * C].rearrange("(j p) -> p j", p=P))

        bv_row = xload.tile([1, C], F32, tag="brow")
        nc.sync.dma_start(bv_row, ba[None, 2 * C:])
        bv_b = const.tile([P, C], F32)
        nc.gpsimd.partition_broadcast(bv_b, bv_row)

        bp_row = xload.tile([1, C], F32, tag="brow")
        nc.sync.dma_start(bp_row, bp[None, :])
        bp_b = const.tile([P, C], F32)
        nc.gpsimd.partition_broadcast(bp_b, bp_row)

        masks = const.tile([P, NMASK, P], VDT)
        nc.sync.dma_start(masks, mk)

        xT = big.tile([P, C // P, TQ], SDT, tag="xT")
        for tt in range(NKT):
            xrow = xload.tile([P, C], F32, tag="xrow")
            nc.sync.dma_start(xrow, xh[tt * P:(tt + 1) * P, :])
            for ct in range(C // P):
                pt = ps128.tile([P, P], F32, tag="ps128")
                nc.tensor.transpose(pt, xrow[:, ct * P:(ct + 1) * P], ident)
                if (tt + ct) % 2 == 0:
                    nc.vector.tensor_copy(xT[:, ct, tt * P:(tt + 1) * P], pt)
                else:
                    nc.scalar.activation(xT[:, ct, tt * P:(tt + 1) * P], pt,
                                         Ident, bias=0.0, scale=1.0)

        QT = big.tile([P, 8, CHUNK], SDT, tag="QT")
        KT = big.tile([P, 8, TQ], SDT, tag="KT")
        VS = big.tile([P, NKT, H, D + 2], VDT, tag="VS")
        ones_h = const.tile([P, NKT * H], F32)
        nc.gpsimd.memset(ones_h, 1.0)
        nc.vector.tensor_copy(
            VS[:, :, :, D], ones_h.rearrange("p (t h) -> p t h", h=H))
        nc.vector.tensor_copy(
            VS[:, :, :, D + 1], ones_h.rearrange("p (t h) -> p t h", h=H))

        for jg in range(2):
            wts = []
            for ct in range(C // P):
                wt = wpool.tile([P, 512], SDT, tag="wchunk")
                nc.sync.dma_start(
                    wt, wa[ct * P:(ct + 1) * P, jg * 512:(jg + 1) * 512])
                wts.append(wt)
            for jl in range(4):
                jt = jg * 4 + jl
                ps = ps512.tile([P, CHUNK], F32, tag="ps512")
                for ct in range(C // P):
                    nc.tensor.matmul(
                        ps,
                        wts[ct][:, jl * P:(jl + 1) * P],
                        xT[:, ct, HALO:TQ],
                        start=(ct == 0), stop=(ct == C // P - 1))
                nc.scalar.activation(QT[:, jt, :], ps, Ident,
                                     bias=bqk[:, jt:jt + 1], scale=1.0)

        for vc in range(2):
            wts = []
            for ct in range(C // P):
                wt = wpool.tile([P, 512], SDT, tag="wchunk")
                nc.sync.dma_start(
                    wt, wa[ct * P:(ct + 1) * P,
                           2 * C + vc * 512: 2 * C + (vc + 1) * 512])
                wts.append(wt)
            for tt in range(NKT):
                ps = ps512.tile([P, 512], F32, tag="ps512")
                for ct in range(C // P):
                    nc.tensor.matmul(
                        ps,
                        xT[:, ct, tt * P:(tt + 1) * P],
                        wts[ct],
                        start=(ct == 0), stop=(ct == C // P - 1))
                nc.vector.tensor_tensor(
                    VS[:, tt, vc * 8:(vc + 1) * 8, 0:D],
                    ps.rearrange("p (h d) -> p h d", d=D),
                    bv_b[:, vc * 512:(vc + 1) * 512]
                        .rearrange("p (h d) -> p h d", d=D),
                    ADD)

        for jg in range(2):
            wts = []
            for ct in range(C // P):
                wt = wpool.tile([P, 512], SDT, tag="wchunk")
                nc.sync.dma_start(
                    wt, wa[ct * P:(ct + 1) * P,
                           C + jg * 512: C + (jg + 1) * 512])
                wts.append(wt)
            for jl in range(4):
                jt = jg * 4 + jl
                for half in range(2):
                    ps = ps384.tile([P, 384], F32, tag="psk")
                    for ct in range(C // P):
                        nc.tensor.matmul(
                            ps,
                            wts[ct][:, jl * P:(jl + 1) * P],
                            xT[:, ct, half * 384:(half + 1) * 384],
                            start=(ct == 0), stop=(ct == C // P - 1))
                    nc.scalar.activation(
                        KT[:, jt, half * 384:(half + 1) * 384], ps, Ident,
                        bias=bqk[:, 8 + jt: 9 + jt], scale=1.0)

        outT = big.tile([P, 8, CHUNK], VDT, tag="outT")
        scale = 1.0 / np.sqrt(D)
        for hp in range(H // 2):
            pair = [small.tile([P, P], F32, tag="pair", name=f"pair{hp}_{i}")
                    for i in range(NQT)]
            for hh in range(2):
                h = 2 * hp + hh
                p0 = hh * 64
                qt_h = QT[p0:p0 + 64, hp, :]
                kt_h = KT[p0:p0 + 64, hp, :]

                slabs = []
                for kt in range(NKT):
                    qlo = max(0, kt - 2)
                    qhi = min(NQT - 1, kt)
                    nq = (qhi - qlo + 1) * P
                    ps = ps384.tile([P, 384], F32, tag="ps384")
                    nc.tensor.matmul(
                        ps[:, :nq],
                        kt_h[:, kt * P:(kt + 1) * P],
                        qt_h[:, qlo * P: qlo * P + nq],
                        start=True, stop=True)
                    slab = slabp.tile([P, 384], VDT, tag="slab")
                    nc.scalar.activation(slab[:, :nq], ps[:, :nq], Exp,
                                         bias=0.0, scale=float(scale))
                    slabs.append(slab)

                for mi, (kt, qt) in enumerate(MASK_PAIRS):
                    qoff = (qt - max(0, kt - 2)) * P
                    nc.vector.tensor_mul(
                        slabs[kt][:, qoff:qoff + P],
                        slabs[kt][:, qoff:qoff + P],
                        masks[:, mi, :])

                for qt in range(NQT):
                    pav = ps128.tile([P, D + 2], F32, tag="ps128")
                    for i, kt in enumerate(range(qt, qt + 3)):
                        qoff = (qt - max(0, kt - 2)) * P
                        nc.tensor.matmul(
                            pav,
                            slabs[kt][:, qoff:qoff + P],
                            VS[:, kt, h, :],
                            start=(i == 0), stop=(i == 2))
                    rcp = small.tile([P, 1], F32, tag="rcp")
                    nc.vector.reciprocal(rcp, pav[:, D:D + 1])
                    nc.vector.tensor_scalar_mul(
                        pair[qt][:, p0:p0 + 64], pav[:, 0:D], rcp)

            for qt in range(NQT):
                pt = ps128.tile([P, P], F32, tag="ps128")
                nc.tensor.transpose(pt, pair[qt], ident)
                nc.vector.tensor_copy(outT[:, hp, qt * P:(qt + 1) * P], pt)

        for oc in range(2):
            wts = []
            for hp in range(8):
                wt = wpool.tile([P, 512], VDT, tag="wchunk")
                nc.sync.dma_start(
                    wt, wp[hp * P:(hp + 1) * P, oc * 512:(oc + 1) * 512])
                wts.append(wt)
            for tb in range(NQT):
                ps = ps512.tile([P, 512], F32, tag="ps512")
                for hp in range(8):
                    nc.tensor.matmul(
                        ps,
                        outT[:, hp, tb * P:(tb + 1) * P],
                        wts[hp],
                        start=(hp == 0), stop=(hp == 7))
                ysb = yout.tile([P, 512], F32, tag="ysb")
                nc.vector.tensor_tensor(
                    ysb, ps, bp_b[:, oc * 512:(oc + 1) * 512], ADD)
                nc.sync.dma_start(
                    y[tb * P:(tb + 1) * P, oc * 512:(oc + 1) * 512], ysb)

    nc.compile()
    return nc


def _get_module():
    global _MOD
    if _MOD is None:
        _MOD = _build_module()
    return _MOD


def _mask_tiles(chunk_start: int) -> np.ndarray:
    out = np.zeros((P, NMASK, P), np.float32)
    kk = np.arange(P)[:, None]
    qq = np.arange(P)[None, :]
    for mi, (kt, qt) in enumerate(MASK_PAIRS):
        key_abs = chunk_start - HALO + kt * P + kk
        q_abs = chunk_start + qt * P + qq
        valid = (key_abs <= q_abs) & (key_abs >= q_abs - WIN) & (key_abs >= 0)
        out[:, mi, :] = valid.astype(np.float32)
    return out


def _in_maps(x, W_attn, b_attn, W_proj, b_proj):
    sdt, vdt = _np_dt(SCORE_DT), _np_dt(VALUE_DT)
    wa = np.ascontiguousarray(np.asarray(W_attn, np.float32).astype(sdt))
    wpp = np.ascontiguousarray(np.asarray(W_proj, np.float32).astype(vdt))
    ba = np.ascontiguousarray(b_attn, np.float32)
    bpp = np.ascontiguousarray(b_proj, np.float32)
    maps = []
    for c in range(NCORES):
        b, k = divmod(c, NCORES // B)
        t0 = k * CHUNK
        xhalo = np.zeros((TQ, C), np.float32)
        lo = t0 - HALO
        src_lo = max(0, lo)
        xhalo[src_lo - lo:, :] = x[b, src_lo: t0 + CHUNK]
        maps.append({
            "xh": xhalo,
            "wa": wa,
            "ba": ba,
            "wp": wpp,
            "bp": bpp,
            "mk": _mask_tiles(t0).astype(vdt),
        })
    return maps


def _run(inputs, trace=False, trace_kwargs=None):
    from concourse import bass_utils

    nc = _get_module()
    maps = _in_maps(**inputs)
    res = bass_utils.run_bass_kernel_spmd(
        nc, maps, core_ids=list(range(NCORES)),
        trace=trace, **(trace_kwargs or {}))
    out = np.empty((B, T, C), np.float32)
    for c in range(NCORES):
        b, k = divmod(c, NCORES // B)
        out[b, k * CHUNK:(k + 1) * CHUNK] = res.results[c]["y"]
    return out, res


def kernel(x, W_attn, b_attn, W_proj, b_proj):
    inputs = dict(x=np.asarray(x, np.float32), W_attn=W_attn, b_attn=b_attn,
                  W_proj=W_proj, b_proj=b_proj)
    out, _ = _run(inputs)
    return out


# revision 12
# speedup vs baseline: 1.4191x; 1.0761x over previous
import os

os.environ.setdefault("MYCRO_LOCAL_CACHE", "1")

import numpy as np

B, T, C = 2, 2048, 1024
H, D = 16, 64
WIN = 256
NCORES = 8
CHUNK = 512
HALO = 256
TQ = CHUNK + HALO
P = 128

NQT = CHUNK // P
NKT = TQ // P

MASK_PAIRS = [(0, 0), (1, 1), (2, 2), (3, 3),
              (1, 0),
              (2, 0), (3, 1), (4, 2), (5, 3)]
NMASK = len(MASK_PAIRS)

SCORE_DT = os.environ.get("KERNEL_SCORE_DT", "bf16")
VALUE_DT = os.environ.get("KERNEL_VALUE_DT", "bf16")

_MOD = None


def _np_dt(name):
    if name == "bf16":
        import ml_dtypes
        return np.dtype(ml_dtypes.bfloat16)
    return np.dtype(np.float32)


def _build_module():
    import concourse.bacc as bacc
    import concourse.mybir as mybir
    import concourse.tile as tile
    from concourse.masks import make_identity
    from contextlib import ExitStack

    F32 = mybir.dt.float32
    SDT = mybir.dt.bfloat16 if SCORE_DT == "bf16" else mybir.dt.float32r
    VDT = mybir.dt.bfloat16 if VALUE_DT == "bf16" else mybir.dt.float32r

    nc = bacc.Bacc(
        "TRN2",
        target_bir_lowering=False,
        debug=False,
        enable_asserts=False,
        num_devices=NCORES,
    )

    xh = nc.dram_tensor("xh", [TQ, C], F32, kind="ExternalInput").ap()
    wa = nc.dram_tensor("wa", [C, 3 * C], SDT, kind="ExternalInput").ap()
    ba = nc.dram_tensor("ba", [3 * C], F32, kind="ExternalInput").ap()
    wp = nc.dram_tensor("wp", [C, C], VDT, kind="ExternalInput").ap()
    bp = nc.dram_tensor("bp", [C], F32, kind="ExternalInput").ap()
    mk = nc.dram_tensor("mk", [P, NMASK, P], VDT, kind="ExternalInput").ap()
    y = nc.dram_tensor("y", [CHUNK, C], F32, kind="ExternalOutput").ap()

    Exp = mybir.ActivationFunctionType.Exp
    Ident = mybir.ActivationFunctionType.Identity
    ADD = mybir.AluOpType.add

    with tile.TileContext(nc) as tc, ExitStack() as ctx:
        const = ctx.enter_context(tc.tile_pool(name="const", bufs=1))
        xload = ctx.enter_context(tc.tile_pool(name="xload", bufs=4))
        big = ctx.enter_context(tc.tile_pool(name="big", bufs=1))
        wpool = ctx.enter_context(tc.tile_pool(name="wpool", bufs=16))
        slabp = ctx.enter_context(tc.tile_pool(name="slabp", bufs=16))
        small = ctx.enter_context(tc.tile_pool(name="small", bufs=12))
        yout = ctx.enter_context(tc.tile_pool(name="yout", bufs=4))
        ps512 = ctx.enter_context(tc.tile_pool(name="ps512", bufs=3, space="PSUM"))
        ps384 = ctx.enter_context(tc.tile_pool(name="ps384", bufs=3, space="PSUM"))
        ps128 = ctx.enter_context(tc.tile_pool(name="ps128", bufs=2, space="PSUM"))

        ident = const.tile([P, P], F32)
        make_identity(nc, ident)

        bqk = const.tile([P, 16], F32)
        with nc.allow_non_contiguous_dma(reason="tiny bias rearrange"):
            nc.sync.dma_start(bqk, ba[: 2 * C].rearrange("(j p) -> p j", p=P))

        bv_row = xload.tile([1, C], F32, tag="brow")
        nc.sync.dma_start(bv_row, ba[None, 2 * C:])
        bv_b = const.tile([P, C], F32)
        nc.gpsimd.partition_broadcast(bv_b, bv_row)

        bp_row = xload.tile([1, C], F32, tag="brow")
        nc.sync.dma_start(bp_row, bp[None, :])
        bp_b = const.tile([P, C], F32)
        nc.gpsimd.partition_broadcast(bp_b, bp_row)

        masks = const.tile([P, NMASK, P], VDT)
        nc.sync.dma_start(masks, mk)

        warm = const.tile([P, 512], SDT)
        nc.gpsimd.memset(warm, 0.0)
        for wi in range(12):
            wps = ps512.tile([P, 512], F32, tag="ps512", name=f"wps{wi}")
            nc.tensor.matmul(wps, warm[:, :P], warm, start=True, stop=True)

        xT = big.tile([P, C // P, TQ], SDT, tag="xT")
        for tt in range(NKT):
            xrow = xload.tile([P, C], F32, tag="xrow")
            nc.sync.dma_start(xrow, xh[tt * P:(tt + 1) * P, :])
            for ct in range(C // P):
                pt = ps128.tile([P, P], F32, tag="ps128")
                nc.tensor.transpose(pt, xrow[:, ct * P:(ct + 1) * P], ident)
                if (tt + ct) % 2 == 0:
                    nc.vector.tensor_copy(xT[:, ct, tt * P:(tt + 1) * P], pt)
                else:
                    nc.scalar.activation(xT[:, ct, tt * P:(tt + 1) * P], pt,
                                         Ident, bias=0.0, scale=1.0)

        QT = big.tile([P, 8, CHUNK], SDT, tag="QT")
        KT = big.tile([P, 8, TQ], SDT, tag="KT")
        VS = big.tile([P, NKT, H, D + 2], VDT, tag="VS")
        ones_h = const.tile([P, NKT * H], F32)
        nc.gpsimd.memset(ones_h, 1.0)
        nc.vector.tensor_copy(
            VS[:, :, :, D], ones_h.rearrange("p (t h) -> p t h", h=H))
        nc.vector.tensor_copy(
            VS[:, :, :, D + 1], ones_h.rearrange("p (t h) -> p t h", h=H))

        for jg in range(2):
            wts = []
            for ct in range(C // P):
                wt = wpool.tile([P, 512], SDT, tag="wchunk")
                nc.sync.dma_start(
                    wt, wa[ct * P:(ct + 1) * P, jg * 512:(jg + 1) * 512])
                wts.append(wt)
            for jl in range(4):
                jt = jg * 4 + jl
                ps = ps512.tile([P, CHUNK], F32, tag="ps512")
                for ct in range(C // P):
                    nc.tensor.matmul(
                        ps,
                        wts[ct][:, jl * P:(jl + 1) * P],
                        xT[:, ct, HALO:TQ],
                        start=(ct == 0), stop=(ct == C // P - 1))
                nc.scalar.activation(QT[:, jt, :], ps, Ident,
                                     bias=bqk[:, jt:jt + 1], scale=1.0)

        for vc in range(2):
            wts = []
            for ct in range(C // P):
                wt = wpool.tile([P, 512], SDT, tag="wchunk")
                nc.sync.dma_start(
                    wt, wa[ct * P:(ct + 1) * P,
                           2 * C + vc * 512: 2 * C + (vc + 1) * 512])
                wts.append(wt)
            for tt in range(NKT):
                ps = ps512.tile([P, 512], F32, tag="ps512")
                for ct in range(C // P):
                    nc.tensor.matmul(
                        ps,
                        xT[:, ct, tt * P:(tt + 1) * P],
                        wts[ct],
                        start=(ct == 0), stop=(ct == C // P - 1))
                nc.vector.tensor_tensor(
                    VS[:, tt, vc * 8:(vc + 1) * 8, 0:D],
                    ps.rearrange("p (h d) -> p h d", d=D),
                    bv_b[:, vc * 512:(vc + 1) * 512]
                        .rearrange("p (h d) -> p h d", d=D),
                    ADD)

        outT = big.tile([P, 8, CHUNK], VDT, tag="outT")
        scale = 1.0 / np.sqrt(D)
        mask_by_kt = {}
        for mi, (kt, qt) in enumerate(MASK_PAIRS):
            mask_by_kt.setdefault(kt, []).append((mi, qt))

        def emit_attention_pair(hp):
            pair = [small.tile([P, P], F32, tag="pair", name=f"pair{hp}_{i}")
                    for i in range(NQT)]
            slabs2 = [[], []]
            for kt in range(NKT):
                qlo = max(0, kt - 2)
                qhi = min(NQT - 1, kt)
                nq = (qhi - qlo + 1) * P
                pss = []
                for hh in range(2):
                    p0 = hh * 64
                    ps = ps384.tile([P, 384], F32, tag="ps384",
                                    name=f"st{hp}_{kt}_{hh}")
                    nc.tensor.matmul(
                        ps[:, :nq],
                        KT[p0:p0 + 64, hp, kt * P:(kt + 1) * P],
                        QT[p0:p0 + 64, hp, qlo * P: qlo * P + nq],
                        start=True, stop=True)
                    pss.append(ps)
                for hh in range(2):
                    ps = pss[hh]
                    for mi, qt in mask_by_kt.get(kt, ()):
                        qoff = (qt - qlo) * P
                        nc.vector.tensor_tensor(
                            ps[:, qoff:qoff + P], ps[:, qoff:qoff + P],
                            masks[:, mi, :], ADD)
                    slab = slabp.tile([P, 384], VDT, tag="slab",
                                      name=f"slab{hp}_{kt}_{hh}")
                    nc.scalar.activation(slab[:, :nq], ps[:, :nq], Exp,
                                         bias=0.0, scale=float(scale))
                    slabs2[hh].append(slab)

            for hh in range(2):
                h = 2 * hp + hh
                p0 = hh * 64
                slabs = slabs2[hh]
                for qt in range(NQT):
                    pav = ps128.tile([P, D + 2], F32, tag="ps128")
                    for i, kt in enumerate(range(qt, qt + 3)):
                        qoff = (qt - max(0, kt - 2)) * P
                        nc.tensor.matmul(
                            pav,
                            slabs[kt][:, qoff:qoff + P],
                            VS[:, kt, h, :],
                            start=(i == 0), stop=(i == 2))
                    rcp = small.tile([P, 1], F32, tag="rcp")
                    nc.vector.reciprocal(rcp, pav[:, D:D + 1])
                    nc.vector.tensor_scalar_mul(
                        pair[qt][:, p0:p0 + 64], pav[:, 0:D], rcp)

            for qt in range(NQT):
                pt = ps128.tile([P, P], F32, tag="ps128")
                nc.tensor.transpose(pt, pair[qt], ident)
                nc.vector.tensor_copy(outT[:, hp, qt * P:(qt + 1) * P], pt)

        for jg in range(2):
            wts = []
            for ct in range(C // P):
                wt = wpool.tile([P, 512], SDT, tag="wchunk")
                nc.sync.dma_start(
                    wt, wa[ct * P:(ct + 1) * P,
                           C + jg * 512: C + (jg + 1) * 512])
                wts.append(wt)
            for jl in range(4):
                jt = jg * 4 + jl
                for half in range(2):
                    ps = ps384.tile([P, 384], F32, tag="ps384")
                    for ct in range(C // P):
                        nc.tensor.matmul(
                            ps,
                            wts[ct][:, jl * P:(jl + 1) * P],
                            xT[:, ct, half * 384:(half + 1) * 384],
                            start=(ct == 0), stop=(ct == C // P - 1))
                    nc.scalar.activation(
                        KT[:, jt, half * 384:(half + 1) * 384], ps, Ident,
                        bias=bqk[:, 8 + jt: 9 + jt], scale=1.0)
                emit_attention_pair(jt)

        for oc in range(2):
            wts = []
            for hp in range(8):
                wt = wpool.tile([P, 512], VDT, tag="wchunk")
                nc.sync.dma_start(
                    wt, wp[hp * P:(hp + 1) * P, oc * 512:(oc + 1) * 512])
                wts.append(wt)
            for tb in range(NQT):
                ps = ps512.tile([P, 512], F32, tag="ps512")
                for hp in range(8):
                    nc.tensor.matmul(
                        ps,
                        outT[:, hp, tb * P:(tb + 1) * P],
                        wts[hp],
                        start=(hp == 0), stop=(hp == 7))
                ysb = yout.tile([P, 512], F32, tag="ysb")
                nc.vector.tensor_tensor(
                    ysb, ps, bp_b[:, oc * 512:(oc + 1) * 512], ADD)
                nc.sync.dma_start(
                    y[tb * P:(tb + 1) * P, oc * 512:(oc + 1) * 512], ysb)

    nc.compile()
    return nc


def _get_module():
    global _MOD
    if _MOD is None:
        _MOD = _build_module()
    return _MOD


def _mask_tiles(chunk_start: int) -> np.ndarray:
    out = np.zeros((P, NMASK, P), np.float32)
    kk = np.arange(P)[:, None]
    qq = np.arange(P)[None, :]
    for mi, (kt, qt) in enumerate(MASK_PAIRS):
        key_abs = chunk_start - HALO + kt * P + kk
        q_abs = chunk_start + qt * P + qq
        valid = (key_abs <= q_abs) & (key_abs >= q_abs - WIN) & (key_abs >= 0)
        out[:, mi, :] = np.where(valid, 0.0, -1e30).astype(np.float32)
    return out


def _in_maps(x, W_attn, b_attn, W_proj, b_proj):
    sdt, vdt = _np_dt(SCORE_DT), _np_dt(VALUE_DT)
    wa = np.ascontiguousarray(np.asarray(W_attn, np.float32).astype(sdt))
    wpp = np.ascontiguousarray(np.asarray(W_proj, np.float32).astype(vdt))
    ba = np.ascontiguousarray(b_attn, np.float32)
    bpp = np.ascontiguousarray(b_proj, np.float32)
    maps = []
    for c in range(NCORES):
        b, k = divmod(c, NCORES // B)
        t0 = k * CHUNK
        xhalo = np.zeros((TQ, C), np.float32)
        lo = t0 - HALO
        src_lo = max(0, lo)
        xhalo[src_lo - lo:, :] = x[b, src_lo: t0 + CHUNK]
        maps.append({
            "xh": xhalo,
            "wa": wa,
            "ba": ba,
            "wp": wpp,
            "bp": bpp,
            "mk": _mask_tiles(t0).astype(vdt),
        })
    return maps


def _run(inputs, trace=False, trace_kwargs=None):
    from concourse import bass_utils

    nc = _get_module()
    maps = _in_maps(**inputs)
    res = bass_utils.run_bass_kernel_spmd(
        nc, maps, core_ids=list(range(NCORES)),
        trace=trace, **(trace_kwargs or {}))
    out = np.empty((B, T, C), np.float32)
    for c in range(NCORES):
        b, k = divmod(c, NCORES // B)
        out[b, k * CHUNK:(k + 1) * CHUNK] = res.results[c]["y"]
    return out, res


def kernel(x, W_attn, b_attn, W_proj, b_proj):
    inputs = dict(x=np.asarray(x, np.float32), W_attn=W_attn, b_attn=b_attn,
                  W_proj=W_proj, b_proj=b_proj)
    out, _ = _run(inputs)
    return out


# revision 14
# speedup vs baseline: 1.5216x; 1.0722x over previous
import os

os.environ.setdefault("MYCRO_LOCAL_CACHE", "1")

import numpy as np

B, T, C = 2, 2048, 1024
H, D = 16, 64
WIN = 256
NCORES = 8
CHUNK = 512
HALO = 256
TQ = CHUNK + HALO
P = 128

NQT = CHUNK // P
NKT = TQ // P

MASK_PAIRS = [(0, 0), (1, 1), (2, 2), (3, 3),
              (1, 0),
              (2, 0), (3, 1), (4, 2), (5, 3)]
NMASK = len(MASK_PAIRS)

SCORE_DT = os.environ.get("KERNEL_SCORE_DT", "bf16")
VALUE_DT = os.environ.get("KERNEL_VALUE_DT", "bf16")

_MODS = {}


def _np_dt(name):
    if name == "bf16":
        import ml_dtypes
        return np.dtype(ml_dtypes.bfloat16)
    return np.dtype(np.float32)


def _build_module(zero_bias):
    import concourse.bacc as bacc
    import concourse.mybir as mybir
    import concourse.tile as tile
    from concourse.masks import make_identity
    from contextlib import ExitStack

    F32 = mybir.dt.float32
    SDT = mybir.dt.bfloat16 if SCORE_DT == "bf16" else mybir.dt.float32r
    VDT = mybir.dt.bfloat16 if VALUE_DT == "bf16" else mybir.dt.float32r

    nc = bacc.Bacc(
        "TRN2",
        target_bir_lowering=False,
        debug=False,
        enable_asserts=False,
        num_devices=NCORES,
    )

    XDT = SDT if SCORE_DT == "bf16" else F32
    xh = nc.dram_tensor("xh", [TQ, C], XDT, kind="ExternalInput").ap()
    wa = nc.dram_tensor("wa", [C, 3 * C], SDT, kind="ExternalInput").ap()
    ba = nc.dram_tensor("ba", [3 * C], F32, kind="ExternalInput").ap()
    wp = nc.dram_tensor("wp", [C, C], VDT, kind="ExternalInput").ap()
    bp = nc.dram_tensor("bp", [C], F32, kind="ExternalInput").ap()
    mk = nc.dram_tensor("mk", [P, NMASK, P], VDT, kind="ExternalInput").ap()
    y = nc.dram_tensor("y", [CHUNK, C], F32, kind="ExternalOutput").ap()

    Exp = mybir.ActivationFunctionType.Exp
    Ident = mybir.ActivationFunctionType.Identity
    ADD = mybir.AluOpType.add

    with tile.TileContext(nc) as tc, ExitStack() as ctx:
        const = ctx.enter_context(tc.tile_pool(name="const", bufs=1))
        xload = ctx.enter_context(tc.tile_pool(name="xload", bufs=4))
        big = ctx.enter_context(tc.tile_pool(name="big", bufs=1))
        wpool = ctx.enter_context(tc.tile_pool(name="wpool", bufs=16))
        slabp = ctx.enter_context(tc.tile_pool(name="slabp", bufs=16))
        small = ctx.enter_context(tc.tile_pool(name="small", bufs=12))
        yout = ctx.enter_context(tc.tile_pool(name="yout", bufs=4))
        ps512 = ctx.enter_context(tc.tile_pool(name="ps512", bufs=3, space="PSUM"))
        ps384 = ctx.enter_context(tc.tile_pool(name="ps384", bufs=3, space="PSUM"))
        ps128 = ctx.enter_context(tc.tile_pool(name="ps128", bufs=2, space="PSUM"))

        ident = const.tile([P, P], F32)
        make_identity(nc, ident)
        if VALUE_DT == "bf16":
            identv = const.tile([P, P], mybir.dt.bfloat16)
            make_identity(nc, identv)
            PAIR_DT = mybir.dt.bfloat16
        else:
            identv = ident
            PAIR_DT = F32

        if not zero_bias:
            bqk = const.tile([P, 16], F32)
            with nc.allow_non_contiguous_dma(reason="tiny bias rearrange"):
                nc.sync.dma_start(
                    bqk, ba[: 2 * C].rearrange("(j p) -> p j", p=P))
            bv_row = xload.tile([1, C], F32, tag="brow")
            nc.sync.dma_start(bv_row, ba[None, 2 * C:])
            bv_b = const.tile([P, C], F32)
            nc.gpsimd.partition_broadcast(bv_b, bv_row)
            bp_row = xload.tile([1, C], F32, tag="brow")
            nc.sync.dma_start(bp_row, bp[None, :])
            bp_b = const.tile([P, C], F32)
            nc.gpsimd.partition_broadcast(bp_b, bp_row)

        masks = const.tile([P, NMASK, P], VDT)
        nc.sync.dma_start(masks, mk)

        warm = const.tile([P, 512], SDT)
        nc.gpsimd.memset(warm, 0.0)
        for wi in range(12):
            wps = ps512.tile([P, 512], F32, tag="ps512", name=f"wps{wi}")
            nc.tensor.matmul(wps, warm[:, :P], warm, start=True, stop=True)

        xT = big.tile([P, C // P, TQ], SDT, tag="xT")
        if SCORE_DT == "bf16":
            for ct in range(C // P):
                nc.sync.dma_start_transpose(
                    xT[:, ct, :], xh[:, ct * P:(ct + 1) * P])
        else:
            for tt in range(NKT):
                xrow = xload.tile([P, C], F32, tag="xrow")
                nc.sync.dma_start(xrow, xh[tt * P:(tt + 1) * P, :])
                for ct in range(C // P):
                    pt = ps128.tile([P, P], F32, tag="ps128")
                    nc.tensor.transpose(pt, xrow[:, ct * P:(ct + 1) * P],
                                        ident)
                    if (tt + ct) % 2 == 0:
                        nc.vector.tensor_copy(
                            xT[:, ct, tt * P:(tt + 1) * P], pt)
                    else:
                        nc.scalar.activation(
                            xT[:, ct, tt * P:(tt + 1) * P], pt,
                            Ident, bias=0.0, scale=1.0)

        QT = big.tile([P, 8, CHUNK], SDT, tag="QT")
        KT = big.tile([P, 8, TQ], SDT, tag="KT")
        VS = big.tile([P, NKT, H, D + 2], VDT, tag="VS")
        ones_h = const.tile([P, NKT * H], F32)
        nc.gpsimd.memset(ones_h, 1.0)
        nc.vector.tensor_copy(
            VS[:, :, :, D], ones_h.rearrange("p (t h) -> p t h", h=H))
        nc.vector.tensor_copy(
            VS[:, :, :, D + 1], ones_h.rearrange("p (t h) -> p t h", h=H))

        for jg in range(2):
            wts = []
            for ct in range(C // P):
                wt = wpool.tile([P, 512], SDT, tag="wchunk")
                nc.sync.dma_start(
                    wt, wa[ct * P:(ct + 1) * P, jg * 512:(jg + 1) * 512])
                wts.append(wt)
            for jl in range(4):
                jt = jg * 4 + jl
                ps = ps512.tile([P, CHUNK], F32, tag="ps512")
                for ct in range(C // P):
                    nc.tensor.matmul(
                        ps,
                        wts[ct][:, jl * P:(jl + 1) * P],
                        xT[:, ct, HALO:TQ],
                        start=(ct == 0), stop=(ct == C // P - 1))
                nc.scalar.activation(
                    QT[:, jt, :], ps, Ident, scale=1.0,
                    bias=0.0 if zero_bias else bqk[:, jt:jt + 1])

        for vc in range(2):
            wts = []
            for ct in range(C // P):
                wt = wpool.tile([P, 512], SDT, tag="wchunk")
                nc.sync.dma_start(
                    wt, wa[ct * P:(ct + 1) * P,
                           2 * C + vc * 512: 2 * C + (vc + 1) * 512])
                wts.append(wt)
            for tt in range(NKT):
                ps = ps512.tile([P, 512], F32, tag="ps512")
                for ct in range(C // P):
                    nc.tensor.matmul(
                        ps,
                        xT[:, ct, tt * P:(tt + 1) * P],
                        wts[ct],
                        start=(ct == 0), stop=(ct == C // P - 1))
                if zero_bias:
                    nc.scalar.activation(
                        VS[:, tt, vc * 8:(vc + 1) * 8, 0:D],
                        ps.rearrange("p (h d) -> p h d", d=D),
                        Ident, bias=0.0, scale=1.0)
                else:
                    nc.vector.tensor_tensor(
                        VS[:, tt, vc * 8:(vc + 1) * 8, 0:D],
                        ps.rearrange("p (h d) -> p h d", d=D),
                        bv_b[:, vc * 512:(vc + 1) * 512]
                            .rearrange("p (h d) -> p h d", d=D),
                        ADD)

        outT = big.tile([P, 8, CHUNK], VDT, tag="outT")
        scale = 1.0 / np.sqrt(D)
        mask_by_kt = {}
        for mi, (kt, qt) in enumerate(MASK_PAIRS):
            mask_by_kt.setdefault(kt, []).append((mi, qt))

        def emit_attention_pair(hp):
            pair = [small.tile([P, P], PAIR_DT, tag="pair",
                               name=f"pair{hp}_{i}")
                    for i in range(NQT)]
            slabs2 = [[], []]
            for kt in range(NKT):
                qlo = max(0, kt - 2)
                qhi = min(NQT - 1, kt)
                nq = (qhi - qlo + 1) * P
                pss = []
                for hh in range(2):
                    p0 = hh * 64
                    ps = ps384.tile([P, 384], F32, tag="ps384",
                                    name=f"st{hp}_{kt}_{hh}")
                    nc.tensor.matmul(
                        ps[:, :nq],
                        KT[p0:p0 + 64, hp, kt * P:(kt + 1) * P],
                        QT[p0:p0 + 64, hp, qlo * P: qlo * P + nq],
                        start=True, stop=True)
                    pss.append(ps)
                for hh in range(2):
                    ps = pss[hh]
                    for mi, qt in mask_by_kt.get(kt, ()):
                        qoff = (qt - qlo) * P
                        nc.vector.tensor_tensor(
                            ps[:, qoff:qoff + P], ps[:, qoff:qoff + P],
                            masks[:, mi, :], ADD)
                    slab = slabp.tile([P, 384], VDT, tag="slab",
                                      name=f"slab{hp}_{kt}_{hh}")
                    nc.scalar.activation(slab[:, :nq], ps[:, :nq], Exp,
                                         bias=0.0, scale=float(scale))
                    slabs2[hh].append(slab)

            for hh in range(2):
                h = 2 * hp + hh
                p0 = hh * 64
                slabs = slabs2[hh]
                for qt in range(NQT):
                    pav = ps128.tile([P, D + 2], F32, tag="ps128")
                    for i, kt in enumerate(range(qt, qt + 3)):
                        qoff = (qt - max(0, kt - 2)) * P
                        nc.tensor.matmul(
                            pav,
                            slabs[kt][:, qoff:qoff + P],
                            VS[:, kt, h, :],
                            start=(i == 0), stop=(i == 2))
                    rcp = small.tile([P, 1], F32, tag="rcp")
                    nc.vector.reciprocal(rcp, pav[:, D:D + 1])
                    nc.vector.tensor_scalar_mul(
                        pair[qt][:, p0:p0 + 64], pav[:, 0:D], rcp)

            for qt in range(NQT):
                pt = ps128.tile([P, P], PAIR_DT, tag="ps128")
                nc.tensor.transpose(pt, pair[qt], identv)
                nc.vector.tensor_copy(outT[:, hp, qt * P:(qt + 1) * P], pt)

        for jg in range(2):
            wts = []
            for ct in range(C // P):
                wt = wpool.tile([P, 512], SDT, tag="wchunk")
                nc.sync.dma_start(
                    wt, wa[ct * P:(ct + 1) * P,
                           C + jg * 512: C + (jg + 1) * 512])
                wts.append(wt)
            for jl in range(4):
                jt = jg * 4 + jl
                for half in range(2):
                    ps = ps384.tile([P, 384], F32, tag="ps384")
                    for ct in range(C // P):
                        nc.tensor.matmul(
                            ps,
                            wts[ct][:, jl * P:(jl + 1) * P],
                            xT[:, ct, half * 384:(half + 1) * 384],
                            start=(ct == 0), stop=(ct == C // P - 1))
                    nc.scalar.activation(
                        KT[:, jt, half * 384:(half + 1) * 384], ps, Ident,
                        scale=1.0,
                        bias=0.0 if zero_bias else bqk[:, 8 + jt: 9 + jt])
                emit_attention_pair(jt)

        for oc in range(2):
            wts = []
            for hp in range(8):
                wt = wpool.tile([P, 512], VDT, tag="wchunk")
                nc.sync.dma_start(
                    wt, wp[hp * P:(hp + 1) * P, oc * 512:(oc + 1) * 512])
                wts.append(wt)
            for tb in range(NQT):
                ps = ps512.tile([P, 512], F32, tag="ps512")
                for hp in range(8):
                    nc.tensor.matmul(
                        ps,
                        outT[:, hp, tb * P:(tb + 1) * P],
                        wts[hp],
                        start=(hp == 0), stop=(hp == 7))
                ysb = yout.tile([P, 512], F32, tag="ysb")
                if zero_bias:
                    nc.scalar.activation(ysb, ps, Ident, bias=0.0, scale=1.0)
                else:
                    nc.vector.tensor_tensor(
                        ysb, ps, bp_b[:, oc * 512:(oc + 1) * 512], ADD)
                nc.sync.dma_start(
                    y[tb * P:(tb + 1) * P, oc * 512:(oc + 1) * 512], ysb)

    nc.compile()
    return nc


def _get_module(zero_bias):
    if zero_bias not in _MODS:
        _MODS[zero_bias] = _build_module(zero_bias)
    return _MODS[zero_bias]


def _mask_tiles(chunk_start: int) -> np.ndarray:
    out = np.zeros((P, NMASK, P), np.float32)
    kk = np.arange(P)[:, None]
    qq = np.arange(P)[None, :]
    for mi, (kt, qt) in enumerate(MASK_PAIRS):
        key_abs = chunk_start - HALO + kt * P + kk
        q_abs = chunk_start + qt * P + qq
        valid = (key_abs <= q_abs) & (key_abs >= q_abs - WIN) & (key_abs >= 0)
        out[:, mi, :] = np.where(valid, 0.0, -1e30).astype(np.float32)
    return out


def _in_maps(x, W_attn, b_attn, W_proj, b_proj):
    sdt, vdt = _np_dt(SCORE_DT), _np_dt(VALUE_DT)
    wa = np.ascontiguousarray(np.asarray(W_attn, np.float32).astype(sdt))
    wpp = np.ascontiguousarray(np.asarray(W_proj, np.float32).astype(vdt))
    ba = np.ascontiguousarray(b_attn, np.float32)
    bpp = np.ascontiguousarray(b_proj, np.float32)
    maps = []
    for c in range(NCORES):
        b, k = divmod(c, NCORES // B)
        t0 = k * CHUNK
        xdt = sdt if SCORE_DT == "bf16" else np.dtype(np.float32)
        xhalo = np.zeros((TQ, C), xdt)
        lo = t0 - HALO
        src_lo = max(0, lo)
        xhalo[src_lo - lo:, :] = x[b, src_lo: t0 + CHUNK].astype(xdt)
        maps.append({
            "xh": xhalo,
            "wa": wa,
            "ba": ba,
            "wp": wpp,
            "bp": bpp,
            "mk": _mask_tiles(t0).astype(vdt),
        })
    return maps


def _run(inputs, trace=False, trace_kwargs=None):
    from concourse import bass_utils

    zero_bias = (not np.any(inputs["b_attn"])) and (not np.any(inputs["b_proj"]))
    nc = _get_module(zero_bias)
    maps = _in_maps(**inputs)
    res = bass_utils.run_bass_kernel_spmd(
        nc, maps, core_ids=list(range(NCORES)),
        trace=trace, **(trace_kwargs or {}))
    out = np.empty((B, T, C), np.float32)
    for c in range(NCORES):
        b, k = divmod(c, NCORES // B)
        out[b, k * CHUNK:(k + 1) * CHUNK] = res.results[c]["y"]
    return out, res


def kernel(x, W_attn, b_attn, W_proj, b_proj):
    inputs = dict(x=np.asarray(x, np.float32), W_attn=W_attn, b_attn=b_attn,
                  W_proj=W_proj, b_proj=b_proj)
    out, _ = _run(inputs)
    return out
